# revision 11
# baseline (speedup 1.0000x reference)
"""Trainium2 Bass kernel for nn_KacLayer_72688026517801.

The layer is: y = x @ W.T + b  +  kac2(vec * kac1(x_2d)), where kac1/kac2 are
seed-derived sequences of 3072 Givens rotations applied to the feature dim.
Both walks are fixed linear maps; with A1/A2 the (constant) walk matrices:

    out = x_2d @ (W.T + (A1 * vec) @ A2) + b = x_2d @ Meff + b

A1/A2 are replayed once on the host from the hardcoded seeds (pure constants);
Meff is a cheap 1024x1024 host prep. The heavy [32768,1024]x[1024,1024] matmul
runs on 8 NeuronCores, data-parallel over token rows (4096 rows/core). Shards
are laid out feature-major ([1024, 4096]) when distributed so each core can
feed the tensor engine's stationary operand directly from DRAM.

Per core: 512-token super-tiles stream in; fp32r matmuls (full PE rate,
~1.5e-4 rel err) accumulate 8 feature-tiles into PSUM; the bias add is fused
into the PSUM->SBUF copy on the DVE; results stream out. Cost-model timeline:
PE ~109us, DMA ~106us, DVE ~45us per core.
"""

import math
from contextlib import ExitStack

import numpy as np

DIM = 1024
SEED = 2024
N_STEPS = math.ceil(math.log2(DIM) * 0.3) * DIM  # 3072
N_CORES = 8
ROWS = 8 * 4096          # flattened tokens
ROWS_PER_CORE = ROWS // N_CORES   # 4096
SUPER = 512              # tokens per super-tile
N_SUPER = ROWS_PER_CORE // SUPER  # 8


def _walk_matrix(seed: int) -> np.ndarray:
    """A such that row-walk(v) == v @ A; float64 accumulation, f32 cos/sin
    (matching the reference's f32 cast of the angles)."""
    rng = np.random.default_rng(seed)
    ii = rng.integers(0, DIM, N_STEPS).astype(np.int32)
    jj = ((ii + rng.integers(1, DIM, N_STEPS)) % DIM).astype(np.int32)
    th = rng.uniform(0.0, 2.0 * np.pi, N_STEPS)
    cs = np.cos(th).astype(np.float32).astype(np.float64)
    sn = np.sin(th).astype(np.float32).astype(np.float64)
    A = np.eye(DIM, dtype=np.float64)
    for i, j, c, s in zip(ii, jj, cs, sn):
        xi = A[:, i].copy()
        xj = A[:, j]
        A[:, i] = c * xi - s * xj
        A[:, j] = s * xi + c * xj
    return A


_A1 = None
_A2 = None
_NC = None


def _get_walks():
    global _A1, _A2
    if _A1 is None:
        _A1 = _walk_matrix(SEED * 2)
        _A2 = _walk_matrix(SEED * 2 + 1)
    return _A1, _A2


def _build_nc():
    """Per-core Bass kernel: out[4096,1024] = xT.T @ mef + b."""
    import concourse.bass as bass
    import concourse.mybir as mybir
    import concourse.tile as tile
    from concourse import bacc

    F32 = mybir.dt.float32
    F32R = mybir.dt.float32r

    nc = bacc.Bacc("TRN2", target_bir_lowering=False)
    xT_d = nc.dram_tensor("xT", [DIM, ROWS_PER_CORE], F32R, kind="ExternalInput")
    mef_d = nc.dram_tensor("mef", [DIM, DIM], F32R, kind="ExternalInput")
    b_d = nc.dram_tensor("bb", [DIM], F32, kind="ExternalInput")
    out_d = nc.dram_tensor("out", [ROWS_PER_CORE, DIM], F32, kind="ExternalOutput")

    with tile.TileContext(nc) as tc, ExitStack() as ctx:
        const = ctx.enter_context(tc.tile_pool(name="const", bufs=1))
        xin = ctx.enter_context(tc.tile_pool(name="xin", bufs=2))
        outp = ctx.enter_context(tc.tile_pool(name="outp", bufs=4))
        pso = ctx.enter_context(tc.tile_pool(name="pso", bufs=4, space="PSUM"))

        xT_t = xT_d.ap().rearrange("(k p) t -> k p t", p=128)
        out_t = out_d.ap().rearrange("(t p) n -> t p n", p=128)

        # Meff resident in SBUF: [128 fi, k-tile, 1024 fo]. Interleave the
        # k-slices of Meff with the first super-tile's x slices so the k-th
        # accumulation step's inputs land just ahead of use instead of the
        # whole 4MB gating the first matmul.
        m_sb = const.tile([128, 8, DIM], F32R)
        mef_t = mef_d.ap().rearrange("(k p) n -> k p n", p=128)
        head = []
        for i in range(1):
            xTs_h = xin.tile([128, 8, SUPER], F32R, tag="x", name=f"xTs_h{i}")
            head.append(xTs_h)
        for k in range(8):
            for s, xTs in enumerate(head):
                nc.sync.dma_start(
                    out=xTs[:, k, :], in_=xT_t[k][:, s * SUPER:(s + 1) * SUPER]
                )
            nc.sync.dma_start(out=m_sb[:, k, :], in_=mef_t[k])

        # bias is first needed by the DVE add, well after the first matmuls
        b_bc = const.tile([128, DIM], F32)
        nc.sync.dma_start(
            out=b_bc,
            in_=bass.AP(tensor=b_d.ap().tensor, offset=0, ap=[[0, 128], [1, DIM]]),
        )

        for s in range(N_SUPER):
            if s < len(head):
                xTs = head[s]
            else:
                xTs = xin.tile([128, 8, SUPER], F32R, tag="x")
                for k in range(8):
                    nc.sync.dma_start(
                        out=xTs[:, k, :], in_=xT_t[k][:, s * SUPER:(s + 1) * SUPER]
                    )
            for tk in range(SUPER // 128):
                po = pso.tile([128, DIM], F32, tag="po")  # 2 PSUM banks
                # k outer / fo-half inner: each stationary xT tile is loaded
                # once and reused for both fo halves (halves the fp32r
                # weight-load traffic into the PE array).
                for k in range(8):
                    for h in range(2):
                        nc.tensor.matmul(
                            po[:, h * 512:(h + 1) * 512],
                            xTs[:, k, tk * 128:(tk + 1) * 128],
                            m_sb[:, k, h * 512:(h + 1) * 512],
                            start=(k == 0),
                            stop=(k == 7),
                        )
                o_sb = outp.tile([128, DIM], F32, tag="o")
                # bias add fused into the PSUM->SBUF copy
                nc.vector.tensor_add(o_sb, po, b_bc)
                nc.sync.dma_start(out=out_t[s * (SUPER // 128) + tk], in_=o_sb)

    nc.compile()
    return nc


def _get_nc():
    global _NC
    if _NC is None:
        _NC = _build_nc()
    return _NC


def kernel(x: np.ndarray, W: np.ndarray, b: np.ndarray, vec: np.ndarray,
           _trace: bool = False):
    from concourse.bass_utils import run_bass_kernel_spmd

    x = np.asarray(x, dtype=np.float32)
    W = np.asarray(W, dtype=np.float32)
    b = np.asarray(b, dtype=np.float32)
    vec = np.asarray(vec, dtype=np.float32)

    A1, A2 = _get_walks()
    nc = _get_nc()

    Meff = (
        W.astype(np.float64).T + (A1 * vec.astype(np.float64)[None, :]) @ A2
    ).astype(np.float32)

    x2 = x.reshape(ROWS, DIM)
    b32 = np.ascontiguousarray(b)
    in_maps = [
        {
            # feature-major shard layout for direct stationary-operand loads
            "xT": np.ascontiguousarray(
                x2[i * ROWS_PER_CORE:(i + 1) * ROWS_PER_CORE].T
            ),
            "mef": Meff,
            "bb": b32,
        }
        for i in range(N_CORES)
    ]
    res = run_bass_kernel_spmd(
        nc, in_maps, core_ids=list(range(N_CORES)), trace=_trace
    )
    out = np.concatenate([r["out"] for r in res.results], axis=0)
    out = out.reshape(x.shape).astype(np.float32)
    if _trace:
        kernel.last_results = res
    return out


# revision 12
# speedup vs baseline: 1.0104x; 1.0104x over previous
"""Trainium2 Bass kernel for nn_KacLayer_72688026517801.

The layer is: y = x @ W.T + b  +  kac2(vec * kac1(x_2d)), where kac1/kac2 are
seed-derived sequences of 3072 Givens rotations applied to the feature dim.
Both walks are fixed linear maps; with A1/A2 the (constant) walk matrices:

    out = x_2d @ (W.T + (A1 * vec) @ A2) + b = x_2d @ Meff + b

A1/A2 are replayed once on the host from the hardcoded seeds (pure constants);
Meff is a cheap 1024x1024 host prep. The heavy [32768,1024]x[1024,1024] matmul
runs on 8 NeuronCores, data-parallel over token rows (4096 rows/core). Shards
are laid out feature-major ([1024, 4096]) when distributed so each core can
feed the tensor engine's stationary operand directly from DRAM.

Per core: 512-token super-tiles stream in; fp32r matmuls (full PE rate,
~1.5e-4 rel err) accumulate 8 feature-tiles into PSUM; the bias add is fused
into the PSUM->SBUF copy on the DVE; results stream out. Cost-model timeline:
PE ~109us, DMA ~106us, DVE ~45us per core.
"""

import math
from contextlib import ExitStack

import numpy as np

DIM = 1024
SEED = 2024
N_STEPS = math.ceil(math.log2(DIM) * 0.3) * DIM  # 3072
N_CORES = 8
ROWS = 8 * 4096          # flattened tokens
ROWS_PER_CORE = ROWS // N_CORES   # 4096
SUPER = 512              # tokens per super-tile
N_SUPER = ROWS_PER_CORE // SUPER  # 8


def _walk_matrix(seed: int) -> np.ndarray:
    """A such that row-walk(v) == v @ A; float64 accumulation, f32 cos/sin
    (matching the reference's f32 cast of the angles)."""
    rng = np.random.default_rng(seed)
    ii = rng.integers(0, DIM, N_STEPS).astype(np.int32)
    jj = ((ii + rng.integers(1, DIM, N_STEPS)) % DIM).astype(np.int32)
    th = rng.uniform(0.0, 2.0 * np.pi, N_STEPS)
    cs = np.cos(th).astype(np.float32).astype(np.float64)
    sn = np.sin(th).astype(np.float32).astype(np.float64)
    A = np.eye(DIM, dtype=np.float64)
    for i, j, c, s in zip(ii, jj, cs, sn):
        xi = A[:, i].copy()
        xj = A[:, j]
        A[:, i] = c * xi - s * xj
        A[:, j] = s * xi + c * xj
    return A


_A1 = None
_A2 = None
_NC = None


def _get_walks():
    global _A1, _A2
    if _A1 is None:
        _A1 = _walk_matrix(SEED * 2)
        _A2 = _walk_matrix(SEED * 2 + 1)
    return _A1, _A2


def _build_nc():
    """Per-core Bass kernel: out[4096,1024] = xT.T @ mef + b."""
    import concourse.bass as bass
    import concourse.mybir as mybir
    import concourse.tile as tile
    from concourse import bacc

    F32 = mybir.dt.float32
    F32R = mybir.dt.float32r

    nc = bacc.Bacc("TRN2", target_bir_lowering=False)
    xT_d = nc.dram_tensor("xT", [DIM, ROWS_PER_CORE], F32R, kind="ExternalInput")
    mef_d = nc.dram_tensor("mef", [DIM, DIM], F32R, kind="ExternalInput")
    b_d = nc.dram_tensor("bb", [DIM], F32, kind="ExternalInput")
    out_d = nc.dram_tensor("out", [ROWS_PER_CORE, DIM], F32, kind="ExternalOutput")

    with tile.TileContext(nc) as tc, ExitStack() as ctx:
        const = ctx.enter_context(tc.tile_pool(name="const", bufs=1))
        xin = ctx.enter_context(tc.tile_pool(name="xin", bufs=2))
        outp = ctx.enter_context(tc.tile_pool(name="outp", bufs=4))
        pso = ctx.enter_context(tc.tile_pool(name="pso", bufs=4, space="PSUM"))

        xT_t = xT_d.ap().rearrange("(k p) t -> k p t", p=128)
        out_t = out_d.ap().rearrange("(t p) n -> t p n", p=128)

        # Meff resident in SBUF: [128 fi, k-tile, 1024 fo]. Interleave the
        # k-slices of Meff with the first super-tile's x slices so the k-th
        # accumulation step's inputs land just ahead of use instead of the
        # whole 4MB gating the first matmul.
        m_sb = const.tile([128, 8, DIM], F32R)
        mef_t = mef_d.ap().rearrange("(k p) n -> k p n", p=128)
        head = []
        for i in range(1):
            xTs_h = xin.tile([128, 8, SUPER], F32R, tag="x", name=f"xTs_h{i}")
            head.append(xTs_h)
        for k in range(8):
            for s, xTs in enumerate(head):
                nc.sync.dma_start(
                    out=xTs[:, k, :], in_=xT_t[k][:, s * SUPER:(s + 1) * SUPER]
                )
            nc.sync.dma_start(out=m_sb[:, k, :], in_=mef_t[k])

        # bias is first needed by the DVE add, well after the first matmuls
        b_bc = const.tile([128, DIM], F32)
        nc.sync.dma_start(
            out=b_bc,
            in_=bass.AP(tensor=b_d.ap().tensor, offset=0, ap=[[0, 128], [1, DIM]]),
        )

        for s in range(N_SUPER):
            if s < len(head):
                xTs = head[s]
            else:
                xTs = xin.tile([128, 8, SUPER], F32R, tag="x")
                for k in range(8):
                    nc.sync.dma_start(
                        out=xTs[:, k, :], in_=xT_t[k][:, s * SUPER:(s + 1) * SUPER]
                    )
            for tk in range(SUPER // 128):
                po = pso.tile([128, DIM], F32, tag="po")  # 2 PSUM banks
                # k outer / fo-half inner: each stationary xT tile is loaded
                # once and reused for both fo halves (halves the fp32r
                # weight-load traffic into the PE array).
                for k in range(8):
                    for h in range(2):
                        nc.tensor.matmul(
                            po[:, h * 512:(h + 1) * 512],
                            xTs[:, k, tk * 128:(tk + 1) * 128],
                            m_sb[:, k, h * 512:(h + 1) * 512],
                            start=(k == 0),
                            stop=(k == 7),
                        )
                o_sb = outp.tile([128, DIM], F32, tag="o")
                # bias add fused into the PSUM->SBUF copy
                nc.vector.tensor_add(o_sb, po, b_bc)
                # stores on the ACT HWDGE queue set: keeps them off the
                # sync-engine queues that feed the x/Meff input stream
                nc.scalar.dma_start(out=out_t[s * (SUPER // 128) + tk], in_=o_sb)

    nc.compile()
    return nc


def _get_nc():
    global _NC
    if _NC is None:
        _NC = _build_nc()
    return _NC


def kernel(x: np.ndarray, W: np.ndarray, b: np.ndarray, vec: np.ndarray,
           _trace: bool = False):
    from concourse.bass_utils import run_bass_kernel_spmd

    x = np.asarray(x, dtype=np.float32)
    W = np.asarray(W, dtype=np.float32)
    b = np.asarray(b, dtype=np.float32)
    vec = np.asarray(vec, dtype=np.float32)

    A1, A2 = _get_walks()
    nc = _get_nc()

    Meff = (
        W.astype(np.float64).T + (A1 * vec.astype(np.float64)[None, :]) @ A2
    ).astype(np.float32)

    x2 = x.reshape(ROWS, DIM)
    b32 = np.ascontiguousarray(b)
    in_maps = [
        {
            # feature-major shard layout for direct stationary-operand loads
            "xT": np.ascontiguousarray(
                x2[i * ROWS_PER_CORE:(i + 1) * ROWS_PER_CORE].T
            ),
            "mef": Meff,
            "bb": b32,
        }
        for i in range(N_CORES)
    ]
    res = run_bass_kernel_spmd(
        nc, in_maps, core_ids=list(range(N_CORES)), trace=_trace
    )
    out = np.concatenate([r["out"] for r in res.results], axis=0)
    out = out.reshape(x.shape).astype(np.float32)
    if _trace:
        kernel.last_results = res
    return out


# revision 13
# speedup vs baseline: 1.0165x; 1.0060x over previous
"""Trainium2 Bass kernel for nn_KacLayer_72688026517801.

The layer is: y = x @ W.T + b  +  kac2(vec * kac1(x_2d)), where kac1/kac2 are
seed-derived sequences of 3072 Givens rotations applied to the feature dim.
Both walks are fixed linear maps; with A1/A2 the (constant) walk matrices:

    out = x_2d @ (W.T + (A1 * vec) @ A2) + b = x_2d @ Meff + b

A1/A2 are replayed once on the host from the hardcoded seeds (pure constants);
Meff is a cheap 1024x1024 host prep. The heavy [32768,1024]x[1024,1024] matmul
runs on 8 NeuronCores, data-parallel over token rows (4096 rows/core). Shards
are laid out feature-major ([1024, 4096]) when distributed so each core can
feed the tensor engine's stationary operand directly from DRAM.

Per core: 512-token super-tiles stream in; fp32r matmuls (full PE rate,
~1.5e-4 rel err) accumulate 8 feature-tiles into PSUM; the bias add is fused
into the PSUM->SBUF copy on the DVE; results stream out. Cost-model timeline:
PE ~109us, DMA ~106us, DVE ~45us per core.
"""

import math
from contextlib import ExitStack

import numpy as np

DIM = 1024
SEED = 2024
N_STEPS = math.ceil(math.log2(DIM) * 0.3) * DIM  # 3072
N_CORES = 8
ROWS = 8 * 4096          # flattened tokens
ROWS_PER_CORE = ROWS // N_CORES   # 4096
SUPER = 512              # tokens per super-tile
N_SUPER = ROWS_PER_CORE // SUPER  # 8


def _walk_matrix(seed: int) -> np.ndarray:
    """A such that row-walk(v) == v @ A; float64 accumulation, f32 cos/sin
    (matching the reference's f32 cast of the angles)."""
    rng = np.random.default_rng(seed)
    ii = rng.integers(0, DIM, N_STEPS).astype(np.int32)
    jj = ((ii + rng.integers(1, DIM, N_STEPS)) % DIM).astype(np.int32)
    th = rng.uniform(0.0, 2.0 * np.pi, N_STEPS)
    cs = np.cos(th).astype(np.float32).astype(np.float64)
    sn = np.sin(th).astype(np.float32).astype(np.float64)
    A = np.eye(DIM, dtype=np.float64)
    for i, j, c, s in zip(ii, jj, cs, sn):
        xi = A[:, i].copy()
        xj = A[:, j]
        A[:, i] = c * xi - s * xj
        A[:, j] = s * xi + c * xj
    return A


_A1 = None
_A2 = None
_NC = None


def _get_walks():
    global _A1, _A2
    if _A1 is None:
        _A1 = _walk_matrix(SEED * 2)
        _A2 = _walk_matrix(SEED * 2 + 1)
    return _A1, _A2


def _build_nc():
    """Per-core Bass kernel: out[4096,1024] = xT.T @ mef + b."""
    import concourse.bass as bass
    import concourse.mybir as mybir
    import concourse.tile as tile
    from concourse import bacc

    F32 = mybir.dt.float32
    F32R = mybir.dt.float32r

    nc = bacc.Bacc("TRN2", target_bir_lowering=False)
    xT_d = nc.dram_tensor("xT", [DIM, ROWS_PER_CORE], F32R, kind="ExternalInput")
    mef_d = nc.dram_tensor("mef", [DIM, DIM], F32R, kind="ExternalInput")
    b_d = nc.dram_tensor("bb", [DIM], F32, kind="ExternalInput")
    out_d = nc.dram_tensor("out", [ROWS_PER_CORE, DIM], F32, kind="ExternalOutput")

    with tile.TileContext(nc) as tc, ExitStack() as ctx:
        const = ctx.enter_context(tc.tile_pool(name="const", bufs=1))
        xin = ctx.enter_context(tc.tile_pool(name="xin", bufs=2))
        outp = ctx.enter_context(tc.tile_pool(name="outp", bufs=4))
        pso = ctx.enter_context(tc.tile_pool(name="pso", bufs=4, space="PSUM"))

        xT_t = xT_d.ap().rearrange("(k p) t -> k p t", p=128)
        out_t = out_d.ap().rearrange("(t p) n -> t p n", p=128)

        # Meff resident in SBUF: [128 fi, k-tile, 1024 fo]. Interleave the
        # k-slices of Meff with the first super-tile's x slices so the k-th
        # accumulation step's inputs land just ahead of use instead of the
        # whole 4MB gating the first matmul.
        m_sb = const.tile([128, 8, DIM], F32R)
        mef_t = mef_d.ap().rearrange("(k p) n -> k p n", p=128)
        head = []
        for i in range(1):
            xTs_h = xin.tile([128, 8, SUPER], F32R, tag="x", name=f"xTs_h{i}")
            head.append(xTs_h)
        for k in range(8):
            for s, xTs in enumerate(head):
                nc.sync.dma_start(
                    out=xTs[:, k, :], in_=xT_t[k][:, s * SUPER:(s + 1) * SUPER]
                )
            nc.sync.dma_start(out=m_sb[:, k, :], in_=mef_t[k])

        # bias is first needed by the DVE add, well after the first matmuls
        b_bc = const.tile([128, DIM], F32)
        nc.sync.dma_start(
            out=b_bc,
            in_=bass.AP(tensor=b_d.ap().tensor, offset=0, ap=[[0, 128], [1, DIM]]),
        )

        for s in range(N_SUPER):
            if s < len(head):
                xTs = head[s]
            else:
                xTs = xin.tile([128, 8, SUPER], F32R, tag="x")
                for k in range(8):
                    nc.sync.dma_start(
                        out=xTs[:, k, :], in_=xT_t[k][:, s * SUPER:(s + 1) * SUPER]
                    )
            for tk in range(SUPER // 128):
                # independent single-bank PSUM tiles per fo-half: finer
                # recycling granularity (a half frees as soon as its own
                # add reads it), and the h0 store overlaps the h1 add.
                po0 = pso.tile([128, 512], F32, tag="po0")
                po1 = pso.tile([128, 512], F32, tag="po1")
                pos = (po0, po1)
                # k outer / fo-half inner: each stationary xT tile is loaded
                # once and reused for both fo halves (halves the fp32r
                # weight-load traffic into the PE array).
                for k in range(8):
                    for h in range(2):
                        nc.tensor.matmul(
                            pos[h],
                            xTs[:, k, tk * 128:(tk + 1) * 128],
                            m_sb[:, k, h * 512:(h + 1) * 512],
                            start=(k == 0),
                            stop=(k == 7),
                        )
                o_sb = outp.tile([128, DIM], F32, tag="o")
                for h in range(2):
                    # bias add fused into the PSUM->SBUF copy; stores on the
                    # ACT HWDGE queues, off the sync queues feeding the
                    # x/Meff input stream
                    nc.vector.tensor_add(
                        o_sb[:, h * 512:(h + 1) * 512], pos[h],
                        b_bc[:, h * 512:(h + 1) * 512])
                    nc.scalar.dma_start(
                        out=out_t[s * (SUPER // 128) + tk][:, h * 512:(h + 1) * 512],
                        in_=o_sb[:, h * 512:(h + 1) * 512])

    nc.compile()
    return nc


def _get_nc():
    global _NC
    if _NC is None:
        _NC = _build_nc()
    return _NC


def kernel(x: np.ndarray, W: np.ndarray, b: np.ndarray, vec: np.ndarray,
           _trace: bool = False):
    from concourse.bass_utils import run_bass_kernel_spmd

    x = np.asarray(x, dtype=np.float32)
    W = np.asarray(W, dtype=np.float32)
    b = np.asarray(b, dtype=np.float32)
    vec = np.asarray(vec, dtype=np.float32)

    A1, A2 = _get_walks()
    nc = _get_nc()

    Meff = (
        W.astype(np.float64).T + (A1 * vec.astype(np.float64)[None, :]) @ A2
    ).astype(np.float32)

    x2 = x.reshape(ROWS, DIM)
    b32 = np.ascontiguousarray(b)
    in_maps = [
        {
            # feature-major shard layout for direct stationary-operand loads
            "xT": np.ascontiguousarray(
                x2[i * ROWS_PER_CORE:(i + 1) * ROWS_PER_CORE].T
            ),
            "mef": Meff,
            "bb": b32,
        }
        for i in range(N_CORES)
    ]
    res = run_bass_kernel_spmd(
        nc, in_maps, core_ids=list(range(N_CORES)), trace=_trace
    )
    out = np.concatenate([r["out"] for r in res.results], axis=0)
    out = out.reshape(x.shape).astype(np.float32)
    if _trace:
        kernel.last_results = res
    return out
